# revision 4
# baseline (speedup 1.0000x reference)
"""Trainium2 Bass kernel for a 20-layer WaveNet-style dilated conv stack.

Strategy: data-parallel over batch (B=8 -> 8 NeuronCores, one batch element
per core).  Per core the full residual stream x [C=128, T=8192] lives in SBUF
(double buffered, left-padded with 2*512 zeros so every causal dilated tap is
a plain in-bounds slice).  Each layer is evaluated in 16 time-tiles of 512:

  PE   : 6 accumulating f32r matmuls (3 taps x {tanh,sigmoid} halves) into
         PSUM, then skip and residual projection matmuls from z.
  ACT  : tanh/sigmoid applied directly from PSUM with the conv bias folded in.
  GPSIMD: z = tanh * sigmoid (keeps DVE under the PE roofline).
  DVE  : skip bias + PSUM->SBUF move, and the fused
         (out_psum + bout) + x_old residual update (scalar_tensor_tensor).

The PE instruction stream is software-pipelined: projection matmuls of tile n
are emitted after conv matmuls of tile n+2 so the PE never waits on the
ACT->GPSIMD z chain.  Weights for layer i+1 are DMA-prefetched during layer i.
"""

import numpy as np
from collections import deque

import concourse.bacc as bacc
import concourse.mybir as mybir
import concourse.tile as tile
from concourse import bass_utils

DILATIONS = [1, 2, 4, 8, 16, 32, 64, 128, 256, 512] * 2
L = len(DILATIONS)
C = 128      # residual channels
S = 128      # skip channels
KS = 3       # conv kernel size
B = 8        # batch == number of cores
T = 8192     # timesteps
TW = 512     # time-tile width (one PSUM bank of fp32)
NT = T // TW
PAD = (KS - 1) * max(DILATIONS)  # 1024: left zero-pad covering max receptive field

f32 = mybir.dt.float32
f32r = mybir.dt.float32r
AOP = mybir.AluOpType
AF = mybir.ActivationFunctionType

SKEW = 2  # tiles between conv emission and projection emission

_NC_CACHE = {}


def _build_nc(reps: int = 1):
    """Trace + schedule + compile the Bass program (identical on all cores)."""
    nc = bacc.Bacc("TRN2", target_bir_lowering=False, debug=False, num_devices=B)

    # host supplies x left-padded with PAD zeros; the pad region also seeds
    # the second ping-pong buffer's pad (memset can't write f32r)
    x_d = nc.dram_tensor("x_in", [C, PAD + T], f32r, kind="ExternalInput").ap()
    wc_d = nc.dram_tensor("wconv_t", [L, C, KS * 2 * C], f32r, kind="ExternalInput").ap()
    wp_d = nc.dram_tensor("wproj_t", [L, C, 2 * C], f32r, kind="ExternalInput").ap()
    b_d = nc.dram_tensor("biases", [C, L * 4], f32, kind="ExternalInput").ap()
    xo_d = nc.dram_tensor("x_out", [C, T], f32, kind="ExternalOutput").ap()
    sk_d = nc.dram_tensor("skips_out", [L, S, T], f32, kind="ExternalOutput").ap()

    with tile.TileContext(nc) as tc:
        with (
            tc.tile_pool(name="const", bufs=1) as const,
            tc.tile_pool(name="wpool", bufs=2) as wpool,
            tc.tile_pool(name="work", bufs=3) as work,
            tc.tile_pool(name="psum", bufs=2, space="PSUM") as psum,
        ):
            bias_sb = const.tile([C, L * 4], f32)
            nc.scalar.dma_start(out=bias_sb[:], in_=b_d[:])

            xa = const.tile([C, PAD + T], f32r)
            xb = const.tile([C, PAD + T], f32r)
            nc.sync.dma_start(out=xa[:], in_=x_d[:])
            nc.sync.dma_start(out=xb[:, 0:PAD], in_=x_d[:, 0:PAD])

            # flat (rep, layer) schedule with next-layer weight prefetch
            steps = [(r, i) for r in range(reps) for i in range(L)]
            wtiles = {}

            def load_weights(idx):
                r, i = steps[idx]
                wc_t = wpool.tile([C, KS * 2 * C], f32r, tag="wc")
                wp_t = wpool.tile([C, 2 * C], f32r, tag="wp")
                nc.scalar.dma_start(out=wc_t[:], in_=wc_d[i])
                nc.scalar.dma_start(out=wp_t[:], in_=wp_d[i])
                wtiles[idx] = (wc_t, wp_t)

            load_weights(0)
            pending = deque()

            def emit_conv(i, n, xin, wc_t):
                d = DILATIONS[i]
                base = PAD + n * TW
                a_ps = psum.tile([C, TW], f32, tag="a")
                g_ps = psum.tile([C, TW], f32, tag="g")
                for k in range(KS):
                    rhs = xin[:, base - (KS - 1 - k) * d: base - (KS - 1 - k) * d + TW]
                    nc.tensor.matmul(a_ps[:], wc_t[:, k * 2 * C: k * 2 * C + C], rhs,
                                     start=(k == 0), stop=(k == KS - 1))
                for k in range(KS):
                    rhs = xin[:, base - (KS - 1 - k) * d: base - (KS - 1 - k) * d + TW]
                    nc.tensor.matmul(g_ps[:], wc_t[:, k * 2 * C + C: (k + 1) * 2 * C], rhs,
                                     start=(k == 0), stop=(k == KS - 1))
                t_sb = work.tile([C, TW], f32, tag="t")
                s_sb = work.tile([C, TW], f32, tag="s")
                nc.scalar.activation(t_sb[:], a_ps[:], AF.Tanh,
                                     bias=bias_sb[:, 4 * i: 4 * i + 1])
                nc.scalar.activation(s_sb[:], g_ps[:], AF.Sigmoid,
                                     bias=bias_sb[:, 4 * i + 1: 4 * i + 2])
                z_sb = work.tile([C, TW], f32r, tag="z")
                nc.gpsimd.tensor_mul(z_sb[:], t_sb[:], s_sb[:])
                return z_sb

            def emit_proj(i, n, xin, xout, wp_t, z_sb):
                t0 = n * TW
                base = PAD + t0
                sk_ps = psum.tile([C, TW], f32, tag="sk")
                nc.tensor.matmul(sk_ps[:], wp_t[:, C:2 * C], z_sb[:], start=True, stop=True)
                sk_sb = work.tile([C, TW], f32, tag="sko")
                nc.vector.tensor_scalar_add(sk_sb[:], sk_ps[:],
                                            bias_sb[:, 4 * i + 2: 4 * i + 3])
                nc.sync.dma_start(out=sk_d[i, :, t0:t0 + TW], in_=sk_sb[:])
                if i < L - 1:
                    o_ps = psum.tile([C, TW], f32, tag="o")
                    nc.tensor.matmul(o_ps[:], wp_t[:, 0:C], z_sb[:], start=True, stop=True)
                    nc.vector.scalar_tensor_tensor(
                        xout[:, base:base + TW], o_ps[:],
                        bias_sb[:, 4 * i + 3: 4 * i + 4],
                        xin[:, base:base + TW], op0=AOP.add, op1=AOP.add)
                else:
                    nc.vector.tensor_add(xout[:, base:base + TW], z_sb[:],
                                         xin[:, base:base + TW])
                    nc.sync.dma_start(out=xo_d[:, t0:t0 + TW],
                                      in_=xout[:, base:base + TW].bitcast(f32))

            for idx, (r, i) in enumerate(steps):
                xin = xa if i % 2 == 0 else xb
                xout = xb if i % 2 == 0 else xa
                wc_t, wp_t = wtiles.pop(idx)
                if idx + 1 < len(steps):
                    load_weights(idx + 1)
                for n in range(NT):
                    z_sb = emit_conv(i, n, xin, wc_t)
                    pending.append((i, n, xin, xout, wp_t, z_sb))
                    if len(pending) > SKEW:
                        emit_proj(*pending.popleft())
            while pending:
                emit_proj(*pending.popleft())

    nc.compile()
    return nc


def _prep_params(Wconv, bconv, Wout, bout, Wskip, bskip):
    Wconv = np.asarray(Wconv, dtype=np.float32)
    bconv = np.asarray(bconv, dtype=np.float32)
    Wout = np.asarray(Wout, dtype=np.float32)
    bout = np.asarray(bout, dtype=np.float32)
    Wskip = np.asarray(Wskip, dtype=np.float32)
    bskip = np.asarray(bskip, dtype=np.float32)

    # lhsT layouts: contraction (input channel) on partitions
    wct = np.ascontiguousarray(
        Wconv.transpose(0, 2, 3, 1).reshape(L, C, KS * 2 * C))
    wpt = np.ascontiguousarray(
        np.concatenate([Wout.transpose(0, 2, 1), Wskip.transpose(0, 2, 1)], axis=2))

    biases = np.zeros((C, L * 4), dtype=np.float32)
    for i in range(L):
        biases[:, 4 * i + 0] = bconv[i, :C]
        biases[:, 4 * i + 1] = bconv[i, C:]
        biases[:, 4 * i + 2] = bskip[i]
        if i < L - 1:
            biases[:, 4 * i + 3] = bout[i]
    return wct, wpt, biases


def kernel(x, Wconv, bconv, Wout, bout, Wskip, bskip):
    x = np.asarray(x, dtype=np.float32)
    wct, wpt, biases = _prep_params(Wconv, bconv, Wout, bout, Wskip, bskip)

    if 1 not in _NC_CACHE:
        _NC_CACHE[1] = _build_nc(reps=1)
    nc = _NC_CACHE[1]

    x_pad = np.zeros((B, C, PAD + T), dtype=np.float32)
    x_pad[:, :, PAD:] = x
    in_maps = [
        {"x_in": x_pad[b], "wconv_t": wct, "wproj_t": wpt, "biases": biases}
        for b in range(B)
    ]
    res = bass_utils.run_bass_kernel_spmd(nc, in_maps, core_ids=list(range(B)))
    x_full = np.stack([res.results[b]["x_out"] for b in range(B)], axis=0)
    skips = np.stack([res.results[b]["skips_out"] for b in range(B)], axis=1)
    return x_full, skips
